# revision 1
# baseline (speedup 1.0000x reference)
"""Distributed statevector Hadamard-gate kernel for 8 TRN2 NeuronCores.

Problem: y = U @ x where U = kron_{i=0..23}(M if i in (0,5,10,15,20) else I2),
x is a 2^24-amplitude complex64 statevector (qudit 0 = most significant axis),
M is the 2x2 Hadamard (real-valued).

Strategy
--------
M is real, so real/imag parts transform independently -> treat x as a float32
stream (interleaved re,im; bit-strides of qubit axes double).

Shard across 8 cores by qubits (1,2,3) (non-gate axes) -> every gate is local
to a core; no collectives. Per core: a 2^22-float slab whose bit layout is

  s = q0*2^21 q4*2^20 q5*2^19 q6*2^18 q7*2^17 q8*2^16 q9*2^15 q10*2^14
      q11*2^13 q12*2^12 q13*2^11 q14*2^10 q15*2^9 | q16..q23,reim (512-run)

On-chip layout: partition index p = q10*64 + q15*32 + q0*16 + q4*8 + q5*4
+ q6*2 + q7.  Gates on q0,q5,q10,q15 then become ONE 128x128 matmul with a
host-precomputed kron matrix L (entries +-s^5, the 5th gate's scale folded
in).  The q20 gate is a free-axis add/sub butterfly on the vector engine.
Single HBM pass: DMA-in -> DVE butterfly -> PE matmul (fp32) -> ACT copy
PSUM->SBUF -> DMA-out, pipelined over 16 x 1MB chunks per core.
"""

import math
import sys
import types

import numpy as np

import concourse.bass as bass
import concourse.mybir as mybir
from concourse.tile import TileContext
from concourse.bass_utils import run_bass_kernel_spmd


def _ensure_axon_hooks():
    """bass_utils' trace path does `from antenv.axon_hooks import ...`
    unconditionally; some images ship an `antenv` without that submodule,
    which would crash tracing.  Synthesize it (and register the ctypes NTFF
    hook when available) so tracing degrades gracefully instead.
    """
    try:
        import antenv.axon_hooks  # noqa: F401

        return
    except ImportError:
        pass
    try:
        import antenv
    except ImportError:
        return
    mod = types.ModuleType("antenv.axon_hooks")
    mod._hook = None

    def set_axon_ntff_profile_hook(hook):
        mod._hook = hook

    def get_axon_ntff_profile_hook():
        return mod._hook

    mod.set_axon_ntff_profile_hook = set_axon_ntff_profile_hook
    mod.get_axon_ntff_profile_hook = get_axon_ntff_profile_hook
    sys.modules["antenv.axon_hooks"] = mod
    antenv.axon_hooks = mod
    try:
        from trn_agent_boot.trn_boot import _ntff_profile_via_ctypes

        hook = _ntff_profile_via_ctypes("/opt/axon/libaxon_pjrt.so")
        if hook is not None:
            mod._hook = hook
    except Exception:
        pass


_ensure_axon_hooks()


def _legalize_waits(bir: dict) -> dict:
    """This image's walrus accepts only ONE sync-wait per TPB/DMA
    instruction; Tile emits up to ~4.  Hoist all but the last wait of each
    instruction into standalone EventSemaphore ops on the same engine,
    placed immediately before it — semantically identical (the engine
    blocks on them in program order).
    """
    for f in bir.get("functions", []):
        for b in f.get("blocks", []):
            out = []
            for i in b["instructions"]:
                si = i.get("sync_info") or {}
                waits = si.get("on_wait") or []
                if len(waits) > 1:
                    for k, wt in enumerate(waits[:-1]):
                        out.append({
                            "debug": i.get("debug", 0),
                            "engine": i["engine"],
                            "ins": [], "outs": [],
                            "name": f"hoistwait_{i['name']}_{k}",
                            "opcode": "EventSemaphore",
                            "sync_info": {"on_update": [], "on_wait": [wt]},
                        })
                    si["on_wait"] = [waits[-1]]
                out.append(i)
            b["instructions"] = out
    return bir


def _install_legalizer():
    import json as _json

    orig = bass.Bass.to_json_bytes
    if getattr(bass.Bass, "_wait_legalizer_installed", False):
        return

    def to_json_bytes(self, *a, **kw):
        raw = orig(self, *a, **kw)
        try:
            return _json.dumps(_legalize_waits(_json.loads(raw))).encode()
        except Exception:
            return raw

    bass.Bass.to_json_bytes = to_json_bytes
    bass.Bass._wait_legalizer_installed = True


_install_legalizer()

N_CORES = 8

_NC_CACHE: dict = {}

# set by kernel(): the BassKernelResults of the last run (exec_time_ns when
# run with BASS_TRACE=1) — used by the local test harness only
LAST_RESULT = None


def _build_nc(S: int, bfly):
    """Build the SPMD Bass program for one core.

    S: log2 of per-core slab float count (22 for complex64 input).
    bfly: ("had",) for add/sub butterfly (scale folded into L), or
          ("gen", a, b, c, d) for a generic real 2x2 q20 gate.
    """
    RUN = 1 << (S - 13)  # contiguous run (q16..q23[,reim]): 512 (cplx) / 256
    CHUNK_FREE = 2 * RUN  # per-partition free elems per chunk (q15, run)
    NCHUNKS = 32  # chunk bits: q9,q11,q12,q13,q14
    L_SUB = RUN // 32  # q21..q23[,reim] size below the q20 bit
    fp = mybir.dt.float32

    nc = bass.Bass()
    x = nc.declare_dram_parameter("x", [1 << S], fp, isOutput=False)
    w = nc.declare_dram_parameter("w", [128, 128], fp, isOutput=False)
    y = nc.declare_dram_parameter("y", [1 << S], fp, isOutput=True)

    # slab bits (MSB..LSB): P=(q0 q4 q5 q6 q7 q8), a=q9, t=q10, c=q11,
    # d=q12, e=q13, m=q14, f = (q15 run) contiguous 2*RUN.
    # Partition index p = P*2 + t  ->  DMA is 3-dim: (64, 2, 2*RUN).
    pat = "(P a t c d e m f) -> a c d e m P t f"
    dims = dict(P=64, a=2, t=2, c=2, d=2, e=2, m=2, f=CHUNK_FREE)
    xv = x[:].rearrange(pat, **dims)
    yv = y[:].rearrange(pat, **dims)

    with TileContext(nc) as tc:
        with (
            tc.tile_pool(name="wpool", bufs=1) as wpool,
            # one dedicated slot per chunk: in-DMAs never reuse a slot, so
            # they carry zero semaphore waits (walrus allows only one per
            # DMA pseudo-instruction)
            tc.tile_pool(name="inp", bufs=NCHUNKS) as inp,
            tc.tile_pool(name="bfp", bufs=3) as bfp,
            tc.tile_pool(name="b2p", bufs=3) as b2p,
            tc.tile_pool(name="outp", bufs=3) as outp,
            tc.tile_pool(name="psp", bufs=4, space="PSUM") as psp,
        ):
            wts = wpool.tile([128, 128], fp, tag="wstage")
            nc.sync.dma_start(out=wts[:], in_=w[:])
            # stage via DVE so matmuls' weight dep is on the DVE semaphore
            wt = wpool.tile([128, 128], fp, tag="wmain")
            nc.vector.tensor_copy(wt[:], wts[:])

            for g in range(NCHUNKS):
                ix = ((g >> 4) & 1, (g >> 3) & 1, (g >> 2) & 1, (g >> 1) & 1, g & 1)

                it = inp.tile([128, CHUNK_FREE], fp)
                nc.sync.dma_start(out=it[:], in_=xv[ix])

                # q15 butterfly: free = (q15, run) = (2, RUN)
                bf = bfp.tile([128, CHUNK_FREE], fp)
                iv = it[:].rearrange("p (w l) -> p w l", w=2, l=RUN)
                bv = bf[:].rearrange("p (w l) -> p w l", w=2, l=RUN)
                _bfly_pair(
                    nc, mybir, bfly,
                    bv[:, 0, :], bv[:, 1, :], iv[:, 0, :], iv[:, 1, :],
                )

                # q20 butterfly: free = (q15 q16..q19, q20, low) = (32, 2, L_SUB)
                b2 = b2p.tile([128, CHUNK_FREE], fp)
                jv = bf[:].rearrange("p (m w l) -> p m w l", m=32, w=2, l=L_SUB)
                ov = b2[:].rearrange("p (m w l) -> p m w l", m=32, w=2, l=L_SUB)
                _bfly_pair(
                    nc, mybir, bfly,
                    ov[:, :, 0, :], ov[:, :, 1, :], jv[:, :, 0, :], jv[:, :, 1, :],
                )

                ps = psp.tile([128, CHUNK_FREE], fp)
                ot = outp.tile([128, CHUNK_FREE], fp)
                for j in range(CHUNK_FREE // RUN):
                    sl = slice(j * RUN, (j + 1) * RUN)
                    nc.tensor.matmul(
                        ps[:, sl], wt[:], b2[:, sl], start=True, stop=True
                    )
                # PSUM evacuation on DVE: keeps every matmul dep on one sem
                nc.vector.tensor_copy(ot[:], ps[:])

                nc.sync.dma_start(out=yv[ix], in_=ot[:])
    return nc


def _bfly_pair(nc, mb, bfly, out0, out1, i0, i1):
    """Apply a 2x2 gate to the (i0, i1) pair of equally-shaped views."""
    if bfly[0] == "had":
        nc.vector.tensor_add(out0, i0, i1)
        nc.vector.tensor_sub(out1, i0, i1)
    else:
        _, ga, gb, gc, gd = bfly
        # out0 = ga*x0 + gb*x1 ; out1 = gc*x0 + gd*x1
        nc.vector.tensor_scalar_mul(out0, i0, float(ga))
        nc.vector.scalar_tensor_tensor(
            out0, i1, float(gb), out0, mb.AluOpType.mult, mb.AluOpType.add
        )
        nc.vector.tensor_scalar_mul(out1, i0, float(gc))
        nc.vector.scalar_tensor_tensor(
            out1, i1, float(gd), out1, mb.AluOpType.mult, mb.AluOpType.add
        )


def _get_nc(S: int, bfly):
    key = (S, bfly)
    if key not in _NC_CACHE:
        _NC_CACHE[key] = _build_nc(S, bfly)
    return _NC_CACHE[key]


def _build_L(Mr: np.ndarray, fold_scale: float) -> np.ndarray:
    """128x128 real matrix applying M on partition bits q0, q5, q10.

    Partition index p = q0*64 + q4*32 + q5*16 + q6*8 + q7*4 + q8*2 + q10.
    """
    I2 = np.eye(2, dtype=np.float64)
    L = np.array([[1.0]])
    for F in (Mr, I2, Mr, I2, I2, I2, Mr):  # q0, q4, q5, q6, q7, q8, q10
        L = np.kron(L, F)
    return (L * fold_scale).astype(np.float32)


def kernel(x: np.ndarray, M: np.ndarray) -> np.ndarray:
    x = np.asarray(x)
    M = np.asarray(M)
    n, batch = x.shape
    assert n == 1 << 24 and batch == 1, (n, batch)

    is_complex = np.iscomplexobj(x)
    if is_complex:
        xc = np.ascontiguousarray(x, dtype=np.complex64)
        xf = xc.reshape(-1).view(np.float32)
    else:
        xf = np.ascontiguousarray(x, dtype=np.float32).reshape(-1)
    F = xf.size
    S = int(round(math.log2(F))) - 3  # per-core slab = F/8 floats

    # gate matrix: must be (essentially) real
    Mc = np.asarray(M, dtype=np.complex128)
    assert np.abs(Mc.imag).max() <= 1e-5 * max(np.abs(Mc.real).max(), 1e-30), (
        "complex-valued M is not supported"
    )
    Mr = Mc.real.copy()

    s0 = Mr[0, 0]
    had_form = (
        abs(s0) > 0
        and abs(Mr[0, 1] - s0) <= 1e-6 * abs(s0)
        and abs(Mr[1, 0] - s0) <= 1e-6 * abs(s0)
        and abs(Mr[1, 1] + s0) <= 1e-6 * abs(s0)
    )
    if had_form:
        bfly = ("had",)
        L = _build_L(Mr, fold_scale=s0 * s0)  # two unnormalized butterflies
    else:
        bfly = ("gen", Mr[0, 0], Mr[0, 1], Mr[1, 0], Mr[1, 1])
        L = _build_L(Mr, fold_scale=1.0)
    wT = np.ascontiguousarray(L.T)  # lhsT[k, i] = L[i, k]

    nc = _get_nc(S, bfly if bfly[0] == "had" else bfly)

    # shard by qubits (1,2,3): xf.reshape(2[q0], 8[q1q2q3], F/16)
    xs = xf.reshape(2, 8, F // 16)
    in_maps = [
        {"x": np.ascontiguousarray(xs[:, cid, :]).reshape(-1), "w": wT}
        for cid in range(N_CORES)
    ]
    res = run_bass_kernel_spmd(nc, in_maps, list(range(N_CORES)))
    global LAST_RESULT
    LAST_RESULT = res
    outs = res.results

    yf = np.empty(F, dtype=np.float32)
    ys = yf.reshape(2, 8, F // 16)
    for cid in range(N_CORES):
        ys[:, cid, :] = outs[cid]["y"].reshape(2, F // 16)

    if is_complex:
        return yf.view(np.complex64).reshape(n, batch)
    return yf.reshape(n, batch)



# revision 2
# speedup vs baseline: 2.3087x; 2.3087x over previous
"""Distributed statevector Hadamard-gate kernel for 8 TRN2 NeuronCores.

Problem: y = U @ x where U = kron_{i=0..23}(M if i in (0,5,10,15,20) else I2),
x is a 2^24-amplitude complex64 statevector (qudit 0 = most significant axis),
M is the 2x2 Hadamard (real-valued).

Strategy
--------
M is real, so real/imag parts transform independently -> treat x as a float
stream (interleaved re,im; bit-strides of qubit axes double).  The rel-err
budget (2e-2) dwarfs fp16 rounding (~1e-3), so all HBM traffic is fp16:
half the bytes of fp32 -> half the memory-roofline time.

Shard across 8 cores by qubits (1,2,3) (non-gate axes) -> every gate is local
to a core; no collectives.  While sharding, the host also permutes qubit axes
so each core's 2^22-elem fp16 slab has bit layout

  s = [q0 q4 q5 q9 q10 q14 q15 | q6 q7 q8 q11 q12 | q13 q16..q19 q20 q21..q23 reim]
       '------ partition -----' '---- chunk -----' '------- chunk free -------'

Partition index = (q0 q4 q5 q9 q10 q14 q15): gates on q0,q5,q10,q15 become ONE
128x128 fp16 matmul with W = kron(M,I,M,I,M,I,M) (q20's scale folded in), and
every DMA is plain 2D with 2KB contiguous lines.  The q20 gate is a fp16
add/sub butterfly on the vector engine (2x 16-bit rate) before the matmul;
PSUM is evacuated fp32->fp16 by the scalar engine.  Single HBM pass,
pipelined over 32 x 256KB chunks per core; DMA is the bottleneck engine.
"""

import math
import sys
import types

import numpy as np

import concourse.bass as bass
import concourse.mybir as mybir
from concourse.tile import TileContext
from concourse.bass_utils import run_bass_kernel_spmd


def _ensure_axon_hooks():
    """bass_utils' trace path does `from antenv.axon_hooks import ...`
    unconditionally; some images ship an `antenv` without that submodule,
    which would crash tracing.  Synthesize it (and register the ctypes NTFF
    hook when available) so tracing degrades gracefully instead.
    """
    try:
        import antenv.axon_hooks  # noqa: F401

        return
    except ImportError:
        pass
    try:
        import antenv
    except ImportError:
        return
    mod = types.ModuleType("antenv.axon_hooks")
    mod._hook = None

    def set_axon_ntff_profile_hook(hook):
        mod._hook = hook

    def get_axon_ntff_profile_hook():
        return mod._hook

    mod.set_axon_ntff_profile_hook = set_axon_ntff_profile_hook
    mod.get_axon_ntff_profile_hook = get_axon_ntff_profile_hook
    sys.modules["antenv.axon_hooks"] = mod
    antenv.axon_hooks = mod
    try:
        from trn_agent_boot.trn_boot import _ntff_profile_via_ctypes

        hook = _ntff_profile_via_ctypes("/opt/axon/libaxon_pjrt.so")
        if hook is not None:
            mod._hook = hook
    except Exception:
        pass


_ensure_axon_hooks()


def _legalize_waits(bir: dict) -> dict:
    """This image's walrus accepts only ONE sync-wait per TPB/DMA
    instruction; Tile emits up to ~4.  Hoist all but the last wait of each
    instruction into standalone EventSemaphore ops on the same engine,
    placed immediately before it — semantically identical (the engine
    blocks on them in program order).
    """
    for f in bir.get("functions", []):
        for b in f.get("blocks", []):
            out = []
            for i in b["instructions"]:
                si = i.get("sync_info") or {}
                waits = si.get("on_wait") or []
                if len(waits) > 1:
                    for k, wt in enumerate(waits[:-1]):
                        out.append({
                            "debug": i.get("debug", 0),
                            "engine": i["engine"],
                            "ins": [], "outs": [],
                            "name": f"hoistwait_{i['name']}_{k}",
                            "opcode": "EventSemaphore",
                            "sync_info": {"on_update": [], "on_wait": [wt]},
                        })
                    si["on_wait"] = [waits[-1]]
                out.append(i)
            b["instructions"] = out
    return bir


def _install_legalizer():
    import json as _json

    orig = bass.Bass.to_json_bytes
    if getattr(bass.Bass, "_wait_legalizer_installed", False):
        return

    def to_json_bytes(self, *a, **kw):
        raw = orig(self, *a, **kw)
        try:
            return _json.dumps(_legalize_waits(_json.loads(raw))).encode()
        except Exception:
            return raw

    bass.Bass.to_json_bytes = to_json_bytes
    bass.Bass._wait_legalizer_installed = True


_install_legalizer()

N_CORES = 8

_NC_CACHE: dict = {}

# set by kernel(): the BassKernelResults of the last run (exec_time_ns when
# run with BASS_TRACE=1) — used by the local test harness only
LAST_RESULT = None


def _build_nc(S: int, bfly):
    """Build the SPMD Bass program for one core.

    S: log2 of per-core slab element count (22 for complex64 input).
    bfly: ("had",) for add/sub butterfly (scale folded into W), or
          ("gen", a, b, c, d) for a generic real 2x2 q20 gate.
    """
    RUN = 1 << (S - 13)  # matmul N / PSUM bank: 512 (cplx) / 256
    CHUNK_FREE = 2 * RUN  # per-partition free elems per chunk
    NCHUNKS = 32
    L_SUB = RUN // 32  # q21..q23[,reim] size below the q20 bit
    f16 = mybir.dt.float16
    fp32 = mybir.dt.float32

    nc = bass.Bass()
    x = nc.declare_dram_parameter("x", [1 << S], f16, isOutput=False)
    w = nc.declare_dram_parameter("w", [128, 128], f16, isOutput=False)
    y = nc.declare_dram_parameter("y", [1 << S], f16, isOutput=True)

    # slab = [p(7) | t(5) | f], p = partition, t = chunk, f = chunk free.
    xv = x[:].rearrange("(p t f) -> t p f", p=128, t=NCHUNKS, f=CHUNK_FREE)
    yv = y[:].rearrange("(p t f) -> t p f", p=128, t=NCHUNKS, f=CHUNK_FREE)

    with TileContext(nc) as tc:
        with (
            tc.tile_pool(name="wpool", bufs=1) as wpool,
            tc.tile_pool(name="inp", bufs=10) as inp,
            tc.tile_pool(name="bfp", bufs=6) as bfp,
            tc.tile_pool(name="outp", bufs=8) as outp,
            tc.tile_pool(name="psp", bufs=4, space="PSUM") as psp,
        ):
            wts = wpool.tile([128, 128], f16, tag="wstage")
            nc.sync.dma_start(out=wts[:], in_=w[:])
            # stage via DVE so matmuls' weight dep is on the DVE semaphore
            wt = wpool.tile([128, 128], f16, tag="wmain")
            nc.vector.tensor_copy(wt[:], wts[:])

            for t in range(NCHUNKS):
                it = inp.tile([128, CHUNK_FREE], f16)
                nc.sync.dma_start(out=it[:], in_=xv[t])

                # q20 butterfly in fp16 (2x DVE rate):
                # free = (m, q20, low) = (32, 2, L_SUB)
                bf = bfp.tile([128, CHUNK_FREE], f16)
                iv = it[:].rearrange("p (m w l) -> p m w l", m=32, w=2, l=L_SUB)
                bv = bf[:].rearrange("p (m w l) -> p m w l", m=32, w=2, l=L_SUB)
                _bfly_pair(
                    nc, mybir, bfly,
                    bv[:, :, 0, :], bv[:, :, 1, :], iv[:, :, 0, :], iv[:, :, 1, :],
                )

                # gates on q0,q5,q10,q15 = one 128x128 matmul on the partition dim
                ps = psp.tile([128, CHUNK_FREE], fp32)
                for j in range(CHUNK_FREE // RUN):
                    sl = slice(j * RUN, (j + 1) * RUN)
                    nc.tensor.matmul(
                        ps[:, sl], wt[:], bf[:, sl], start=True, stop=True
                    )

                # PSUM evacuation fp32->fp16 on the (otherwise idle) scalar engine
                ot = outp.tile([128, CHUNK_FREE], f16)
                nc.scalar.copy(ot[:], ps[:])

                nc.sync.dma_start(out=yv[t], in_=ot[:])
    return nc


def _bfly_pair(nc, mb, bfly, out0, out1, i0, i1):
    """Apply a 2x2 gate to the (i0, i1) pair of equally-shaped views."""
    if bfly[0] == "had":
        nc.vector.tensor_add(out0, i0, i1)
        nc.vector.tensor_sub(out1, i0, i1)
    else:
        _, ga, gb, gc, gd = bfly
        # out0 = ga*x0 + gb*x1 ; out1 = gc*x0 + gd*x1
        nc.vector.tensor_scalar_mul(out0, i0, float(ga))
        nc.vector.scalar_tensor_tensor(
            out0, i1, float(gb), out0, mb.AluOpType.mult, mb.AluOpType.add
        )
        nc.vector.tensor_scalar_mul(out1, i0, float(gc))
        nc.vector.scalar_tensor_tensor(
            out1, i1, float(gd), out1, mb.AluOpType.mult, mb.AluOpType.add
        )


def _get_nc(S: int, bfly):
    key = (S, bfly)
    if key not in _NC_CACHE:
        _NC_CACHE[key] = _build_nc(S, bfly)
    return _NC_CACHE[key]


def _build_W(Mr: np.ndarray, fold_scale: float) -> np.ndarray:
    """128x128 real matrix applying M on partition bits q0, q5, q10, q15.

    Partition index p = (q0 q4 q5 q9 q10 q14 q15), MSB first.
    """
    I2 = np.eye(2, dtype=np.float64)
    W = np.array([[1.0]])
    for F in (Mr, I2, Mr, I2, Mr, I2, Mr):  # q0, q4, q5, q9, q10, q14, q15
        W = np.kron(W, F)
    return W * fold_scale


def kernel(x: np.ndarray, M: np.ndarray) -> np.ndarray:
    x = np.asarray(x)
    M = np.asarray(M)
    n, batch = x.shape
    assert n == 1 << 24 and batch == 1, (n, batch)

    is_complex = np.iscomplexobj(x)
    if is_complex:
        xc = np.ascontiguousarray(x, dtype=np.complex64)
        xf = xc.reshape(-1).view(np.float32)
    else:
        xf = np.ascontiguousarray(x, dtype=np.float32).reshape(-1)
    F = xf.size
    S = int(round(math.log2(F))) - 3  # per-core slab elems = F/8
    FD = F >> 16  # contiguous tail (q16..q23[,reim]): 512 (cplx) / 256

    # gate matrix: must be (essentially) real
    Mc = np.asarray(M, dtype=np.complex128)
    assert np.abs(Mc.imag).max() <= 1e-5 * max(np.abs(Mc.real).max(), 1e-30), (
        "complex-valued M is not supported"
    )
    Mr = Mc.real.copy()

    s0 = Mr[0, 0]
    had_form = (
        abs(s0) > 0
        and abs(Mr[0, 1] - s0) <= 1e-6 * abs(s0)
        and abs(Mr[1, 0] - s0) <= 1e-6 * abs(s0)
        and abs(Mr[1, 1] + s0) <= 1e-6 * abs(s0)
    )
    if had_form:
        bfly = ("had",)
        W = _build_W(Mr, fold_scale=s0)  # q20's unnormalized butterfly scale
    else:
        bfly = ("gen", Mr[0, 0], Mr[0, 1], Mr[1, 0], Mr[1, 1])
        W = _build_W(Mr, fold_scale=1.0)
    wT = np.ascontiguousarray(W.T).astype(np.float16)  # lhsT[k, i] = W[i, k]

    nc = _get_nc(S, bfly)

    # fp16 + shard by qubits (1,2,3) + permute (q9 q10),(q14 q15) up into the
    # partition bits.  Full-array dims, MSB->LSB:
    #   (q0, q1q2q3, q4q5, q6q7q8, q9q10, q11q12q13, q14q15, tail)
    xh = xf.astype(np.float16)
    xt = xh.reshape(2, 8, 4, 8, 4, 8, 4, FD).transpose(1, 0, 2, 4, 6, 3, 5, 7)
    xs = np.ascontiguousarray(xt)  # (core, q0, q4q5, q9q10, q14q15, q6q7q8, q11q12q13, tail)
    in_maps = [
        {"x": xs[cid].reshape(-1), "w": wT} for cid in range(N_CORES)
    ]
    res = run_bass_kernel_spmd(nc, in_maps, list(range(N_CORES)))
    global LAST_RESULT
    LAST_RESULT = res
    outs = res.results

    yt = np.empty((8, 2, 4, 4, 4, 8, 8, FD), dtype=np.float16)
    for cid in range(N_CORES):
        yt[cid] = outs[cid]["y"].reshape(2, 4, 4, 4, 8, 8, FD)
    # inverse permute + upcast
    yf = yt.transpose(1, 0, 2, 5, 3, 6, 4, 7).astype(np.float32).reshape(F)

    if is_complex:
        return yf.view(np.complex64).reshape(n, batch)
    return yf.reshape(n, batch)
